# revision 11
# baseline (speedup 1.0000x reference)
"""Trainium2 Bass kernel for nn_DCConv3dKernelUnitPolynomials.

out(r, th, ph) = sum over 30 hydrogen-like orbitals c_i * R_{n,l}(r) * Y_{l,m}(th, ph)
for (n,l,m) with n<=4, l<=min(n-1,3), |m|<=l; positions (4096, 512, 3).

Strategy (8 NeuronCores, data-parallel over OutN):
  out = sum_n E_n(r) * F_n(r, x, y, u),  E_n = exp(-r/n),
  x = sin(th)cos(ph), y = sin(th)sin(ph), u = cos(th); y^2 -> 1-u^2-x^2.
  The 30 coefficients fold (host-side) into scalar weights on:
    - angular combos G_n1 (u,x,y), G_n2 (u2,x2,xu,yu,xy), Qu/Qx/Qy (cubic
      groups), each + a `ones` term carrying the radial-constant weights
    - a small PE-accumulated remainder {e1,e2,e3,e4, rho*e4r3}
  Combos and the final accumulation run on the TensorEngine as float32r
  diagonal matmuls accumulating in PSUM; elementwise products run on
  DVE/GPSIMD; transcendentals on ACT (Sin args range-reduced to [-pi,pi]).

Engines per [128,512] chunk (4 chunks/core):
  ACT: 4 trig + 3 exp + 3 squares; DVE: ~22 tensor_tensor + 4 tensor_scalar
  GPSIMD: 5 tensor_tensor; PE: ~43 diag-matmul terms.
"""
import math
import os

import numpy as np

import concourse.bacc as bacc
import concourse.bass as bass
import concourse.mybir as mybir
import concourse.tile as tile
from concourse.bass_utils import run_bass_kernel_spmd

# ---------------------------------------------------------------- constants
OUT_N, CONV = 4096, 512
N_CORES = 8
ROWS = OUT_N // N_CORES            # 512
ELEMS = ROWS * CONV                # 262144
P = 128
FREE = ELEMS // P                  # 2048
CHUNK = 512
NCH = FREE // CHUNK                # 4
N_Q, K_Q, M_Q = 4, 3, 3

F32 = mybir.dt.float32
F32R = mybir.dt.float32r
F16 = mybir.dt.float16
# Work dtype for elementwise tiles: F32 (V1) or F16 (V2: 2x DVE TT, exact PE reads)
WORK_DT = F32 if os.environ.get("KERNEL_WORK_DT", "f16") == "f32" else F16

# ------------------------------------------------------------ host-side math
def _basis_list():
    basis = []
    for n in range(1, N_Q + 1):
        for k in range(0, min(n, K_Q + 1)):
            for m in range(-k, k + 1):
                if abs(m) <= M_Q:
                    basis.append((n, k, m))
    return basis


def _legendre_coeffs(l):
    c = [0.0] * (l + 1)
    for k in range(l // 2 + 1):
        c[l - 2 * k] = ((-1) ** k * math.comb(l, k) * math.comb(2 * l - 2 * k, l)) / (2.0 ** l)
    return c


def _diff_poly(c):
    return [c[i] * i for i in range(1, len(c))] if len(c) > 1 else [0.0]


def _radial_poly(n, l):
    p, alpha = n - l - 1, 2 * l + 1
    norm = math.sqrt((2.0 / n) ** 3 * math.factorial(p) / (2.0 * n * math.factorial(n + l)))
    s = 2.0 / n
    coeffs = [0.0] * n
    for i in range(p + 1):
        lag_c = ((-1.0) ** i) * math.comb(p + alpha, p - i) / math.factorial(i)
        coeffs[l + i] = norm * (s ** l) * lag_c * (s ** i)
    return coeffs


def _ylm_norm(l, m):
    am = abs(m)
    return math.sqrt((2 * l + 1) / (4.0 * math.pi) * math.factorial(l - am) / math.factorial(l + am))


def _pmul(p1, p2):
    out = {}
    for (a1, b1, c1), v1 in p1.items():
        for (a2, b2, c2), v2 in p2.items():
            k = (a1 + a2, b1 + b2, c1 + c2)
            out[k] = out.get(k, 0.0) + v1 * v2
    return _reduce_y2(out)


def _padd(p1, p2, s=1.0):
    out = dict(p1)
    for k, v in p2.items():
        out[k] = out.get(k, 0.0) + s * v
    return out


def _reduce_y2(p):
    changed = True
    while changed:
        changed = False
        out = {}
        for (a, b, c), v in p.items():
            if c >= 2:
                changed = True
                for k2, v2 in (((0, 0, 0), 1.0), ((2, 0, 0), -1.0), ((0, 2, 0), -1.0)):
                    k = (a + k2[0], b + k2[1], c - 2 + k2[2])
                    out[k] = out.get(k, 0.0) + v * v2
            else:
                out[(a, b, c)] = out.get((a, b, c), 0.0) + v
        p = out
    return p


def _ylm_poly(l, m):
    am = abs(m)
    c = _legendre_coeffs(l)
    for _ in range(am):
        c = _diff_poly(c)
    dP = {(i, 0, 0): v for i, v in enumerate(c) if v != 0.0}
    if am == 0:
        ang = {(0, 0, 0): 1.0}
        pref = _ylm_norm(l, m)
    else:
        re, im = {}, {}
        for j in range(am + 1):
            coef = math.comb(am, j)
            k = (0, am - j, j)
            if j % 4 == 0:
                re[k] = re.get(k, 0.0) + coef
            elif j % 4 == 1:
                im[k] = im.get(k, 0.0) + coef
            elif j % 4 == 2:
                re[k] = re.get(k, 0.0) - coef
            else:
                im[k] = im.get(k, 0.0) - coef
        ang = re if m > 0 else im
        pref = _ylm_norm(l, m) * math.sqrt(2.0) * ((-1.0) ** am)
    return {k: pref * v for k, v in _pmul(dP, ang).items()}


def compute_scalars(coefficients):
    c = np.asarray(coefficients, dtype=np.float64)
    basis = _basis_list()
    idx = {b: i for i, b in enumerate(basis)}

    G = {}
    for i, (n, l, m) in enumerate(basis):
        if l == 0:
            continue
        G[(n, l)] = _padd(G.get((n, l), {}), _ylm_poly(l, m), s=float(c[i]))

    q21 = _radial_poly(2, 1)
    q31 = _radial_poly(3, 1)
    q41 = _radial_poly(4, 1)
    q32 = _radial_poly(3, 2)
    q42 = _radial_poly(4, 2)
    q43 = _radial_poly(4, 3)
    lam21 = q21[1]
    lam31 = (q31[1], q31[2])                  # a + b r
    lam41 = (q41[1], q41[2], q41[3])          # c0 + c1 r + c2 r^2
    lam32 = q32[2]
    lam42 = (q42[2], q42[3])                  # t + s r
    kap43 = q43[3]

    S = {}

    def l1(n, scale):
        g = G.get((n, 1), {})
        return [scale * g.get((1, 0, 0), 0.0), scale * g.get((0, 1, 0), 0.0),
                scale * g.get((0, 0, 1), 0.0)]

    S["G21"] = l1(2, lam21)
    S["G31"] = l1(3, 1.0)
    S["G41"] = l1(4, 1.0)
    S["lam31"] = lam31
    S["lam41"] = lam41
    S["lam42"] = lam42

    g2_consts = {}
    for n, scale, key in ((3, lam32, "G32"), (4, 1.0, "G42")):
        g = G.get((n, 2), {})
        S[key] = [scale * g.get((2, 0, 0), 0.0), scale * g.get((0, 2, 0), 0.0),
                  scale * g.get((1, 1, 0), 0.0), scale * g.get((1, 0, 1), 0.0),
                  scale * g.get((0, 1, 1), 0.0)]
        g2_consts[n] = scale * g.get((0, 0, 0), 0.0)

    g43 = G.get((4, 3), {})
    k = kap43
    S["Qu"] = [k * g43.get((3, 0, 0), 0.0), k * g43.get((1, 2, 0), 0.0),
               k * g43.get((1, 1, 1), 0.0), k * g43.get((1, 0, 0), 0.0)]
    S["Qx"] = [k * g43.get((2, 1, 0), 0.0), k * g43.get((0, 3, 0), 0.0),
               k * g43.get((0, 1, 0), 0.0)]
    S["Qy"] = [k * g43.get((2, 0, 1), 0.0), k * g43.get((0, 2, 1), 0.0),
               k * g43.get((0, 0, 1), 0.0)]
    g43_const = k * g43.get((0, 0, 0), 0.0)

    # l=0 radial aggregate weights
    k00 = _ylm_norm(0, 0)
    Atil = {}
    for n in range(1, 5):
        poly = np.zeros(n)
        poly[: n] += float(c[idx[(n, 0, 0)]]) * k00 * np.array(_radial_poly(n, 0))
        Atil[n] = poly
    Atil[3][2] += g2_consts[3]                       # lam32 already folded
    Atil[4] += g2_consts[4] * np.array(_radial_poly(4, 2))
    Atil[4][3] += g43_const

    w_e1 = Atil[1][0]
    w_e2, w_e2r = Atil[2][0], Atil[2][1]
    w_e3, w_e3r, w_e3r2 = Atil[3]
    w_e4, w_e4r, w_e4r2, w_e4r3 = Atil[4]

    # Fold radial-poly weights into combo `ones` terms (exact, via DVE products):
    #   G21.ones = w_e2r                      (P21 = G21' * e2r)
    #   G31.ones = w_e3r / lam31.a            (rides rho3 = e3r * (a + b r))
    #   G32.ones = w_e3r2 - G31.ones * lam31.b
    #   G41.ones = w_e4r / lam41.c0
    #   G42.ones = (w_e4r2 - G41.ones*c1) / lam42.t
    #   residual rho = w_e4r3 - G41.ones*c2 - G42.ones*lam42.s  (PE term on e4r3)
    S["G21"].append(w_e2r)
    o31 = w_e3r / lam31[0]
    S["G31"].append(o31)
    S["G32"].append(w_e3r2 - o31 * lam31[1])
    o41 = w_e4r / lam41[0]
    S["G41"].append(o41)
    o42 = (w_e4r2 - o41 * lam41[1]) / lam42[0]
    S["G42"].append(o42)
    rho_e4r3 = w_e4r3 - o41 * lam41[2] - o42 * lam42[1]

    S["ACC"] = {"e1": w_e1, "e2": w_e2, "e3": w_e3, "e4": w_e4, "e4r3": rho_e4r3}
    return S


# Diagonal slot layout (program-fixed; values runtime)
DIAG_ORDER = (
    [f"G21_{i}" for i in range(4)] + [f"G31_{i}" for i in range(4)]
    + [f"G41_{i}" for i in range(4)] + [f"G32_{i}" for i in range(6)]
    + [f"G42_{i}" for i in range(6)] + [f"Qu_{i}" for i in range(4)]
    + [f"Qx_{i}" for i in range(3)] + [f"Qy_{i}" for i in range(3)]
    + ["ACC_e1", "ACC_e2", "ACC_e3", "ACC_e4", "ACC_e4r3", "ID"]
)
ND = len(DIAG_ORDER)
DIAG_SLOT = {name: i for i, name in enumerate(DIAG_ORDER)}


def build_diags(S):
    vals = {}
    for key in ("G21", "G31", "G41", "G32", "G42", "Qu", "Qx", "Qy"):
        for i, v in enumerate(S[key]):
            vals[f"{key}_{i}"] = v
    for key, v in S["ACC"].items():
        vals[f"ACC_{key}"] = v
    vals["ID"] = 1.0
    np_dt = np.float32 if WORK_DT == F32 else np.float16
    diags = np.zeros((P, ND * P), dtype=np_dt)
    ii = np.arange(P)
    for name, slot in DIAG_SLOT.items():
        diags[ii, slot * P + ii] = vals[name]
    return diags


# ------------------------------------------------------------ device program
def _emit(nc, tc, drams):
    r_d, th_d, ph_d, dg_d, out_d = drams
    Alu = mybir.AluOpType
    Af = mybir.ActivationFunctionType
    HALF_PI = math.pi / 2.0

    S31 = _radial_poly(3, 1)
    lam31_a, lam31_b = S31[1], S31[2]
    S41 = _radial_poly(4, 1)
    c0, c1, c2 = S41[1], S41[2], S41[3]
    alpha41 = c1 / (2.0 * c2)
    m41 = c0 - c1 * c1 / (4.0 * c2)
    S42 = _radial_poly(4, 2)
    lam42_t, lam42_s = S42[2], S42[3]

    import contextlib
    ctx = contextlib.ExitStack()
    with ctx:
        cpool = ctx.enter_context(tc.tile_pool(name="const", bufs=1))
        diags = cpool.tile([P, ND * P], WORK_DT)
        ones = cpool.tile([P, CHUNK], WORK_DT)
        nc.sync.dma_start(diags[:], dg_d[:])
        nc.vector.memset(ones[:], 1.0)

        def const_col(val, tag):
            tcol = cpool.tile([P, 1], F32, tag=tag, name=tag)
            nc.vector.memset(tcol[:], float(val))
            return tcol

        b_halfpi = const_col(HALF_PI, "b_halfpi")
        b_negpi = const_col(-math.pi, "b_negpi")
        b_alpha41 = const_col(alpha41, "b_alpha41")

        pin = ctx.enter_context(tc.tile_pool(name="pin", bufs=2))
        pr = ctx.enter_context(tc.tile_pool(name="pr", bufs=NCH))
        ptrig = ctx.enter_context(tc.tile_pool(name="ptrig", bufs=NCH))
        ptmp = ctx.enter_context(tc.tile_pool(name="ptmp", bufs=2))
        pe_ = ctx.enter_context(tc.tile_pool(name="pe", bufs=NCH))
        pc = ctx.enter_context(tc.tile_pool(name="pc", bufs=1))
        pprod = ctx.enter_context(tc.tile_pool(name="pprod", bufs=2))
        ppsum = ctx.enter_context(
            tc.tile_pool(name="ppsum", bufs=4, space=bass.MemorySpace.PSUM))
        pout = ctx.enter_context(
            tc.tile_pool(name="pout", bufs=2, space=bass.MemorySpace.PSUM))

        def mmcast(ap):
            return ap

        def dslot(name):
            s = DIAG_SLOT[name]
            return mmcast(diags[:, s * P:(s + 1) * P])

        st = [{} for _ in range(NCH)]

        # ---- phase A: DMA + trig (Sin table) ----
        for ci in range(NCH):
            t = st[ci]
            sl = slice(ci * CHUNK, (ci + 1) * CHUNK)
            r = pr.tile([P, CHUNK], F32, tag="r")
            th = pin.tile([P, CHUNK], F32, tag="th")
            ph = pin.tile([P, CHUNK], F32, tag="ph")
            nc.sync.dma_start(r[:], r_d[:, sl])
            nc.sync.dma_start(th[:], th_d[:, sl])
            nc.sync.dma_start(ph[:], ph_d[:, sl])
            u = ptrig.tile([P, CHUNK], WORK_DT, tag="u")
            stn = ptmp.tile([P, CHUNK], WORK_DT, tag="stn")
            tabs = ptmp.tile([P, CHUNK], WORK_DT, tag="tabs")
            cpt = ptmp.tile([P, CHUNK], WORK_DT, tag="cpt")
            spt = ptmp.tile([P, CHUNK], WORK_DT, tag="spt")
            x = ptrig.tile([P, CHUNK], WORK_DT, tag="x")
            y = ptrig.tile([P, CHUNK], WORK_DT, tag="y")
            # u = cos th = sin(pi/2 - th); stn = -sin th = sin(-th)
            nc.scalar.activation(u[:], th[:], Af.Sin, bias=b_halfpi[:], scale=-1.0)
            nc.scalar.activation(stn[:], th[:], Af.Sin, bias=0.0, scale=-1.0)
            # tabs = |ph - pi|; cpt = cos|ph-pi| = -cos ph; spt = sin(ph-pi) = -sin ph
            nc.scalar.activation(tabs[:], ph[:], Af.Abs, bias=b_negpi[:])
            nc.scalar.activation(cpt[:], tabs[:], Af.Sin, bias=b_halfpi[:], scale=-1.0)
            nc.scalar.activation(spt[:], ph[:], Af.Sin, bias=b_negpi[:], scale=1.0)
            nc.vector.tensor_tensor(x[:], stn[:], cpt[:], Alu.mult)
            nc.vector.tensor_tensor(y[:], stn[:], spt[:], Alu.mult)
            t["r"], t["u"], t["x"], t["y"] = r, u, x, y

        # ---- phase B: exps (Exp table) ----
        for ci in range(NCH):
            t = st[ci]
            e2 = pe_.tile([P, CHUNK], WORK_DT, tag="e2")
            e3 = pe_.tile([P, CHUNK], WORK_DT, tag="e3")
            e4 = pe_.tile([P, CHUNK], WORK_DT, tag="e4")
            nc.scalar.activation(e2[:], t["r"][:], Af.Exp, scale=-0.5)
            nc.scalar.activation(e3[:], t["r"][:], Af.Exp, scale=-1.0 / 3.0)
            nc.scalar.activation(e4[:], t["r"][:], Af.Exp, scale=-0.25)
            t["e2"], t["e3"], t["e4"] = e2, e3, e4

        # ---- phase C: products, combos, accumulation ----
        for ci in range(NCH):
            t = st[ci]
            sl = slice(ci * CHUNK, (ci + 1) * CHUNK)
            r, u, x, y = t["r"], t["u"], t["x"], t["y"]
            e2, e3, e4 = t["e2"], t["e3"], t["e4"]
            if WORK_DT == F32:
                rw = r
            else:
                rw = pc.tile([P, CHUNK], WORK_DT, tag="rw", name="rw")
                nc.vector.tensor_copy(rw[:], r[:])

            def tl(tag, pool=pc):
                return pool.tile([P, CHUNK], WORK_DT, tag=tag, name=tag)

            # squares on ACT (Square lives in every table set)
            u2 = tl("u2"); nc.scalar.activation(u2[:], u[:], Af.Square)
            lam41q = tl("lam41q")
            nc.scalar.activation(lam41q[:], r[:], Af.Square, bias=b_alpha41[:])
            # GPSIMD offload
            x2 = tl("x2"); nc.gpsimd.tensor_tensor(x2[:], x[:], x[:], Alu.mult)
            xy = tl("xy"); nc.gpsimd.tensor_tensor(xy[:], x[:], y[:], Alu.mult)
            xu = tl("xu"); nc.gpsimd.tensor_tensor(xu[:], x[:], u[:], Alu.mult)
            yu = tl("yu"); nc.gpsimd.tensor_tensor(yu[:], y[:], u[:], Alu.mult)
            e1 = tl("e1"); nc.gpsimd.tensor_tensor(e1[:], e2[:], e2[:], Alu.mult)
            # DVE products
            e2r = tl("e2r"); nc.vector.tensor_tensor(e2r[:], e2[:], rw[:], Alu.mult)
            e3r = tl("e3r"); nc.vector.tensor_tensor(e3r[:], e3[:], rw[:], Alu.mult)
            e4r = tl("e4r"); nc.vector.tensor_tensor(e4r[:], e4[:], rw[:], Alu.mult)
            e3r2 = tl("e3r2"); nc.vector.tensor_tensor(e3r2[:], e3r[:], rw[:], Alu.mult)
            e4r2 = tl("e4r2"); nc.vector.tensor_tensor(e4r2[:], e4r[:], rw[:], Alu.mult)
            e4r3 = tl("e4r3"); nc.vector.tensor_tensor(e4r3[:], e4r2[:], rw[:], Alu.mult)
            lam31 = tl("lam31")
            nc.vector.tensor_scalar(lam31[:], r[:], lam31_b, lam31_a, Alu.mult, Alu.add)
            lam41 = tl("lam41")
            nc.vector.tensor_scalar(lam41[:], lam41q[:], c2, m41, Alu.mult, Alu.add)
            lam42 = tl("lam42")
            nc.vector.tensor_scalar(lam42[:], r[:], lam42_s, lam42_t, Alu.mult, Alu.add)
            rho3 = tl("rho3"); nc.vector.tensor_tensor(rho3[:], e3r[:], lam31[:], Alu.mult)
            rho4 = tl("rho4"); nc.vector.tensor_tensor(rho4[:], e4r[:], lam41[:], Alu.mult)
            rho42 = tl("rho42"); nc.vector.tensor_tensor(rho42[:], e4r2[:], lam42[:], Alu.mult)

            # PE combos -> PSUM
            def combo(name, terms):
                ps = ppsum.tile([P, CHUNK], F32, tag="combo", name=name)
                nterm = len(terms)
                for j, (slot, src) in enumerate(terms):
                    nc.tensor.matmul(ps[:], dslot(slot), mmcast(src[:]),
                                     start=(j == 0), stop=(j == nterm - 1))
                return ps

            G21 = combo("G21", [("G21_0", u), ("G21_1", x), ("G21_2", y), ("G21_3", ones)])
            G31 = combo("G31", [("G31_0", u), ("G31_1", x), ("G31_2", y), ("G31_3", ones)])
            G41 = combo("G41", [("G41_0", u), ("G41_1", x), ("G41_2", y), ("G41_3", ones)])
            G32 = combo("G32", [("G32_0", u2), ("G32_1", x2), ("G32_2", xu),
                                ("G32_3", yu), ("G32_4", xy), ("G32_5", ones)])
            G42 = combo("G42", [("G42_0", u2), ("G42_1", x2), ("G42_2", xu),
                                ("G42_3", yu), ("G42_4", xy), ("G42_5", ones)])
            Qu = combo("Qu", [("Qu_0", u2), ("Qu_1", x2), ("Qu_2", xy), ("Qu_3", ones)])
            Qx = combo("Qx", [("Qx_0", u2), ("Qx_1", x2), ("Qx_2", ones)])
            Qy = combo("Qy", [("Qy_0", u2), ("Qy_1", x2), ("Qy_2", ones)])

            def prod(tag, a, b):
                o = pprod.tile([P, CHUNK], WORK_DT, tag=tag, name=tag)
                nc.vector.tensor_tensor(o[:], a[:], b[:], Alu.mult)
                return o

            P21 = prod("P21", G21, e2r)
            P31 = prod("P31", G31, rho3)
            P41 = prod("P41", G41, rho4)
            P32 = prod("P32", G32, e3r2)
            P42 = prod("P42", G42, rho42)
            Au = prod("Au", e4r3, u)
            Ax = prod("Ax", e4r3, x)
            Ay = prod("Ay", e4r3, y)
            PQu = prod("PQu", Qu, Au)
            PQx = prod("PQx", Qx, Ax)
            PQy = prod("PQy", Qy, Ay)

            # final accumulation in PSUM
            out_ps = pout.tile([P, CHUNK], F32, tag="out", name="out_ps")
            acc_terms = [("ACC_e1", e1), ("ACC_e2", e2), ("ACC_e3", e3),
                         ("ACC_e4", e4), ("ACC_e4r3", e4r3),
                         ("ID", P21), ("ID", P31), ("ID", P41), ("ID", P32),
                         ("ID", P42), ("ID", PQu), ("ID", PQx), ("ID", PQy)]
            for j, (slot, src) in enumerate(acc_terms):
                nc.tensor.matmul(out_ps[:], dslot(slot), mmcast(src[:]),
                                 start=(j == 0), stop=(j == len(acc_terms) - 1))
            out_sb = pprod.tile([P, CHUNK], F32, tag="out_sb", name="out_sb")
            nc.scalar.copy(out_sb[:], out_ps[:])
            nc.sync.dma_start(out_d[:, sl], out_sb[:])


_PROGRAM_CACHE = {}


def _build_program():
    if "nc" in _PROGRAM_CACHE:
        return _PROGRAM_CACHE["nc"]
    nc = bacc.Bacc("TRN2", target_bir_lowering=False, debug=False)
    r_d = nc.dram_tensor("r_in", [P, FREE], F32, kind="ExternalInput").ap()
    th_d = nc.dram_tensor("th_in", [P, FREE], F32, kind="ExternalInput").ap()
    ph_d = nc.dram_tensor("ph_in", [P, FREE], F32, kind="ExternalInput").ap()
    dg_d = nc.dram_tensor("diags", [P, ND * P], WORK_DT, kind="ExternalInput").ap()
    out_d = nc.dram_tensor("out", [P, FREE], F32, kind="ExternalOutput").ap()
    with tile.TileContext(nc) as tc:
        _emit(nc, tc, (r_d, th_d, ph_d, dg_d, out_d))
    nc.compile()
    _PROGRAM_CACHE["nc"] = nc
    return nc


# ----------------------------------------------------------------- interface
def make_in_maps(position, coefficients):
    position = np.asarray(position, dtype=np.float32)
    coefficients = np.asarray(coefficients, dtype=np.float32)
    diags = build_diags(compute_scalars(coefficients))
    pos = position.reshape(N_CORES, ROWS, CONV, 3)
    in_maps = []
    for ci in range(N_CORES):
        pc_ = pos[ci].reshape(ELEMS, 3)
        in_maps.append({
            "r_in": np.ascontiguousarray(pc_[:, 0]).reshape(P, FREE),
            "th_in": np.ascontiguousarray(pc_[:, 1]).reshape(P, FREE),
            "ph_in": np.ascontiguousarray(pc_[:, 2]).reshape(P, FREE),
            "diags": diags,
        })
    return in_maps


def assemble_out(results):
    out = np.empty((N_CORES, ROWS, CONV), dtype=np.float32)
    for ci in range(N_CORES):
        out[ci] = results[ci]["out"].reshape(ROWS, CONV)
    return out.reshape(OUT_N, CONV)


def run(position, coefficients, trace=False):
    in_maps = make_in_maps(position, coefficients)
    nc = _build_program()
    res = run_bass_kernel_spmd(nc, in_maps, list(range(N_CORES)), trace=trace)
    return assemble_out(res.results), res


def kernel(position, coefficients):
    out, _ = run(position, coefficients, trace=False)
    return out


if __name__ == "__main__":
    rng = np.random.default_rng(0)
    pos = np.stack([rng.uniform(0, 10, (OUT_N, CONV)),
                    rng.uniform(0, math.pi, (OUT_N, CONV)),
                    rng.uniform(0, 2 * math.pi, (OUT_N, CONV))], axis=-1).astype(np.float32)
    coef = rng.standard_normal(30).astype(np.float32)
    out = kernel(pos, coef)
    print("out", out.shape, out.dtype, float(np.abs(out).max()))
